# revision 14
# baseline (speedup 1.0000x reference)
"""Cross-modal attention Trainium2 kernel (Bass/Tile), data-parallel over batch.

Per core (one batch element):
    q = img @ Wq.T + bq ; k = ts @ Wk.T + bk ; v = ts @ Wv.T + bv
    out = softmax(q @ k.T) @ v

Layout strategy: the host passes PRE-TRANSPOSED operands (imgT/tsT [d, n],
WqT/WkT/WvT [d, e]) so the contraction dim is already on SBUF partitions
when DMA'd in — no PE transposes at all, and the projection phase is a pure
fp32r matmul stream:
  - qT/kT [e%128, ec, n] accumulate dc-outer (4 psum groups live) so the
    first matmuls start as soon as the first 256KB weight/img pieces land.
  - v is produced naturally [j%128, jt, e] in fp16 for the PV matmul.
  - attention per q-tile: scores S[i=128, j=2048] as 2x [128,1024] PSUM
    chunk tiles; per-512 DVE reduce_max overlaps the score matmuls; exp on
    ACT in 4x512 chunks (bias=-rowmax, accum_out=partial sums) written as
    fp16 probs; each 512-chunk is DMA-XBAR-transposed (sync queue) into
    probsT [j, i] the moment it lands, so the PV accumulation (16 fp16
    matmuls) starts after the first chunk, not the whole row.
  - softmax->PV software pipeline lag is 1 q-tile; 1/rowsum is applied to
    the PV result on ACT; out DMA rides the scalar HWDGE queue.
"""

import numpy as np

import concourse.bass as bass
import concourse.mybir as mybir
import concourse.tile as tile
from concourse import bacc
from concourse.bass_utils import run_bass_kernel_spmd

B, NQ, NK, D = 8, 2048, 2048, 512
P = 128
DC = D // P        # 4 contraction chunks
EC = D // P        # 4 output-dim chunks
TQ = NQ // P       # 16 query tiles
TK = NK // P       # 16 key tiles
JC = NK // 512     # 4 key chunks of 512 (scores free dim)
IC = NQ // 512     # 4 query chunks of 512 (projection free dim)

F32 = mybir.dt.float32
F32R = mybir.dt.float32r
F16 = mybir.dt.float16
BF16 = mybir.dt.bfloat16
AX = mybir.AxisListType.X
IDENT_FN = mybir.ActivationFunctionType.Identity
EXP_FN = mybir.ActivationFunctionType.Exp


def build():
    nc = bacc.Bacc(None, target_bir_lowering=False)

    imgT = nc.dram_tensor("imgT", [D, NQ], F32, kind="ExternalInput")
    tsT = nc.dram_tensor("tsT", [D, NK], F32, kind="ExternalInput")
    WqT = nc.dram_tensor("WqT", [D, D], F32, kind="ExternalInput")
    WkT = nc.dram_tensor("WkT", [D, D], F32, kind="ExternalInput")
    WvT = nc.dram_tensor("WvT", [D, D], F32, kind="ExternalInput")
    bq = nc.dram_tensor("bq", [D], F32, kind="ExternalInput")
    bk = nc.dram_tensor("bk", [D], F32, kind="ExternalInput")
    bv = nc.dram_tensor("bv", [D], F32, kind="ExternalInput")
    out = nc.dram_tensor("out", [NQ, D], F32, kind="ExternalOutput")

    with tile.TileContext(nc) as tc:
        with (
            tc.tile_pool(name="const", bufs=1) as const_pool,
            tc.tile_pool(name="big", bufs=1) as big,
        ):
            # biases: bq/bk as [P, EC] (e%128 on partitions), bv replicated
            # [P, D]; tiny descriptors go on the gpsimd SWDGE queue.
            bq_sb = const_pool.tile([P, EC], F32)
            nc.gpsimd.dma_start(bq_sb[:], bq.ap().rearrange("(c p) -> p c", p=P))
            bk_sb = const_pool.tile([P, EC], F32)
            nc.gpsimd.dma_start(bk_sb[:], bk.ap().rearrange("(c p) -> p c", p=P))
            bv_sb = const_pool.tile([P, D], F32)
            nc.gpsimd.dma_start(bv_sb[:], bv.ap().partition_broadcast(P))

            # persistent big operands
            qT = big.tile([P, EC, NQ], F32R)   # qT[p, ec, i] = q[i, ec*128+p]
            kT = big.tile([P, EC, NK], F32R)
            v_sb = big.tile([P, TK, D], F16)   # v_sb[p, jt, e] = v[jt*128+p, e]

            # ---- Phase A: projections (no transposes; dc-outer accumulate).
            # DMA payloads stay plain fp32 (a float32r-typed DMA engages the
            # DMA's fp32r rounding mode, whose per-engine state then corrupts
            # the fp16 XBAR transposes in phase B). The fp32r matmul operands
            # are produced by DVE/ACT round-copies instead (bf16 is too coarse
            # here: the unscaled scores have sigma~22, so near-tie rows flip
            # their argmax under 8-bit-mantissa q/k).
            with (
                tc.tile_pool(name="w16", bufs=1) as wpool,
                tc.tile_pool(name="stage", bufs=3) as stage,
                tc.tile_pool(name="x16", bufs=3) as x16p,
                tc.tile_pool(name="pps", bufs=8, space="PSUM") as pps,
            ):
                def emit_w(wdram, eng, idx):
                    # per-dc 256KB pieces so the first cast can start before
                    # the whole 1MB matrix lands; casts ride DVE/GpSimd so the
                    # two HWDGE queues stay pure-DMA and never stall on sems
                    w32 = stage.tile([P, DC, D], F32, tag="stage", name=f"w32_{idx}")
                    w16 = wpool.tile([P, DC, D], F32R, name=f"w16_{idx}")
                    for dc in range(DC):
                        eng.dma_start(w32[:, dc, :], wdram[dc * P : (dc + 1) * P, :])
                        if dc % 2 == 0:
                            nc.vector.tensor_copy(w16[:, dc, :], w32[:, dc, :])
                        else:
                            nc.gpsimd.tensor_copy(w16[:, dc, :], w32[:, dc, :])
                    return w16

                def emit_x(xdram, ic, eng, idx):
                    s32 = stage.tile([P, DC, 512], F32, tag="stage", name=f"s32_{idx}")
                    s16 = x16p.tile([P, DC, 512], F32R, tag="x16", name=f"s16_{idx}")
                    for dc in range(DC):
                        eng.dma_start(
                            s32[:, dc, :],
                            xdram[dc * P : (dc + 1) * P, ic * 512 : (ic + 1) * 512],
                        )
                        if dc % 2 == 0:
                            nc.vector.tensor_copy(s16[:, dc, :], s32[:, dc, :])
                        else:
                            nc.gpsimd.tensor_copy(s16[:, dc, :], s32[:, dc, :])
                    return s16

                def emit_xproj(s, w_sb, dstT, bias_sb, ic):
                    """dstT[:, :, ic*512:+512] = W.T @ x chunk + bias, dc-outer."""
                    pqs = [
                        pps.tile([P, 512], F32, tag="pps", name=f"pq{ec}")
                        for ec in range(EC)
                    ]
                    for dc in range(DC):
                        for ec in range(EC):
                            nc.tensor.matmul(
                                pqs[ec][:],
                                w_sb[:, dc, ec * P : (ec + 1) * P],
                                s[:, dc, :],
                                start=(dc == 0),
                                stop=(dc == DC - 1),
                            )
                    for ec in range(EC):
                        if ec % 2 == 0:
                            nc.scalar.activation(
                                out=dstT[:, ec, ic * 512 : (ic + 1) * 512],
                                in_=pqs[ec][:],
                                func=IDENT_FN,
                                bias=bias_sb[:, ec : ec + 1],
                                scale=1.0,
                            )
                        else:
                            nc.vector.tensor_scalar_add(
                                dstT[:, ec, ic * 512 : (ic + 1) * 512],
                                pqs[ec][:],
                                bias_sb[:, ec : ec + 1],
                            )

                def emit_v(s, wv_sb, ic):
                    """v rows jt=4ic..4ic+3: v[j,e] = ts_chunk.T @ Wv.T + bv."""
                    for t in range(4):
                        jt = 4 * ic + t
                        pv = pps.tile([P, 512], F32, tag="pps")
                        for dc in range(DC):
                            nc.tensor.matmul(
                                pv[:],
                                s[:, dc, t * P : (t + 1) * P],
                                wv_sb[:, dc, :],
                                start=(dc == 0),
                                stop=(dc == DC - 1),
                            )
                        nc.vector.tensor_add(v_sb[:, jt, :], pv[:], bv_sb[:])

                # DMA schedule: sync queue carries img0-3, ts2, ts3; scalar
                # queue carries Wq, Wk, ts0, Wv, ts1. Arrival order matches
                # consumption order so the PE never starves.
                wq_sb = emit_w(WqT, nc.sync, "q")
                s0 = emit_x(imgT, 0, nc.scalar, "s0")
                s1 = emit_x(imgT, 1, nc.sync, "s1")
                s2 = emit_x(imgT, 2, nc.scalar, "s2")
                emit_xproj(s0, wq_sb, qT, bq_sb, 0)
                s3 = emit_x(imgT, 3, nc.sync, "s3")
                wk_sb = emit_w(WkT, nc.scalar, "k")
                emit_xproj(s1, wq_sb, qT, bq_sb, 1)
                t0 = emit_x(tsT, 0, nc.sync, "t0")
                emit_xproj(s2, wq_sb, qT, bq_sb, 2)
                wv_sb = emit_w(WvT, nc.scalar, "v")
                emit_xproj(s3, wq_sb, qT, bq_sb, 3)

                t1 = emit_x(tsT, 1, nc.scalar, "t1")
                emit_xproj(t0, wk_sb, kT, bk_sb, 0)
                emit_v(t0, wv_sb, 0)
                t2 = emit_x(tsT, 2, nc.sync, "t2")
                emit_xproj(t1, wk_sb, kT, bk_sb, 1)
                emit_v(t1, wv_sb, 1)
                t3 = emit_x(tsT, 3, nc.scalar, "t3")
                emit_xproj(t2, wk_sb, kT, bk_sb, 2)
                emit_v(t2, wv_sb, 2)
                emit_xproj(t3, wk_sb, kT, bk_sb, 3)
                emit_v(t3, wv_sb, 3)

            # ---- Phase B: attention (software-pipelined by one q-tile) ----
            with (
                tc.tile_pool(name="sps", bufs=3, space="PSUM") as sps,
                tc.tile_pool(name="ops", bufs=2, space="PSUM") as ops,
                tc.tile_pool(name="soft", bufs=3) as soft,
                tc.tile_pool(name="outp", bufs=2) as outp,
            ):
                stash = {}

                def emit_scores_softmax(qt):
                    pmax = soft.tile([P, JC], F32, tag="pmax")
                    chunks = []
                    for jc2 in range(2):
                        Sc = sps.tile([P, 1024], F32, tag="S")
                        chunks.append(Sc)
                        for h in range(2):
                            jc = 2 * jc2 + h
                            for ec in range(EC):
                                nc.tensor.matmul(
                                    Sc[:, h * 512 : (h + 1) * 512],
                                    qT[:, ec, qt * P : (qt + 1) * P],
                                    kT[:, ec, jc * 512 : (jc + 1) * 512],
                                    start=(ec == 0),
                                    stop=(ec == EC - 1),
                                )
                            # chunk max overlaps the next chunk's matmuls
                            nc.vector.reduce_max(
                                pmax[:, jc : jc + 1],
                                Sc[:, h * 512 : (h + 1) * 512],
                                axis=AX,
                            )
                    negmax = soft.tile([P, 1], F32, tag="negmax")
                    nc.vector.reduce_max(negmax[:], pmax[:], axis=AX, negate=True)
                    probs = soft.tile([P, NK], F16, tag="probs")
                    rowsum4 = soft.tile([P, JC], F32, tag="rowsum4")
                    probsT = soft.tile([P, TK, P], F16, tag="probsT")
                    for jc in range(JC):
                        nc.scalar.activation(
                            out=probs[:, jc * 512 : (jc + 1) * 512],
                            in_=chunks[jc // 2][:, (jc % 2) * 512 : (jc % 2 + 1) * 512],
                            func=EXP_FN,
                            bias=negmax[:],
                            scale=1.0,
                            accum_out=rowsum4[:, jc : jc + 1],
                        )
                        # transpose each 512-chunk via the DMA XBAR as soon as
                        # its exp lands so PV can start on the first chunk
                        nc.sync.dma_start_transpose(
                            probsT[:, jc * 4 : (jc + 1) * 4, :],
                            probs[:, jc * 512 : (jc + 1) * 512],
                        )
                    stash[qt] = (probsT, rowsum4)

                def emit_pv(qt):
                    probsT, rowsum4 = stash.pop(qt)
                    rowsum = soft.tile([P, 1], F32, tag="rowsum")
                    nc.vector.reduce_sum(rowsum[:], rowsum4[:], axis=AX)
                    recip = soft.tile([P, 1], F32, tag="recip")
                    nc.vector.reciprocal(recip[:], rowsum[:])
                    po = ops.tile([P, D], F32, tag="po")
                    for jt in range(TK):
                        nc.tensor.matmul(
                            po[:],
                            probsT[:, jt, :],
                            v_sb[:, jt, :],
                            start=(jt == 0),
                            stop=(jt == TK - 1),
                        )
                    o_sb = outp.tile([P, D], F32, tag="o")
                    nc.scalar.mul(out=o_sb[:], in_=po[:], mul=recip[:])
                    nc.scalar.dma_start(out[qt * P : (qt + 1) * P, :], o_sb[:])

                for qt in range(TQ):
                    emit_scores_softmax(qt)
                    if qt >= 1:
                        emit_pv(qt - 1)
                emit_pv(TQ - 1)

    nc.compile()
    return nc


_NC_CACHE = None


def _get_nc():
    global _NC_CACHE
    if _NC_CACHE is None:
        _NC_CACHE = build()
    return _NC_CACHE


def run(inputs: dict, trace: bool = False):
    """Run on 8 cores, batch-parallel. Returns (out [B,NQ,D], BassKernelResults)."""
    nc = _get_nc()
    imgT = np.ascontiguousarray(
        np.transpose(np.asarray(inputs["img_feats"], np.float32), (0, 2, 1))
    )
    tsT = np.ascontiguousarray(
        np.transpose(np.asarray(inputs["ts_feats"], np.float32), (0, 2, 1))
    )
    WqT = np.ascontiguousarray(np.asarray(inputs["Wq"], np.float32).T)
    WkT = np.ascontiguousarray(np.asarray(inputs["Wk"], np.float32).T)
    WvT = np.ascontiguousarray(np.asarray(inputs["Wv"], np.float32).T)
    in_maps = []
    for b in range(B):
        in_maps.append(
            {
                "imgT": imgT[b],
                "tsT": tsT[b],
                "WqT": WqT,
                "WkT": WkT,
                "WvT": WvT,
                "bq": np.asarray(inputs["bq"], np.float32),
                "bk": np.asarray(inputs["bk"], np.float32),
                "bv": np.asarray(inputs["bv"], np.float32),
            }
        )
    res = run_bass_kernel_spmd(nc, in_maps, core_ids=list(range(B)), trace=trace)
    full = np.stack([res.results[b]["out"] for b in range(B)], axis=0)
    return full, res


def kernel(**inputs) -> np.ndarray:
    full, _ = run(inputs, trace=False)
    return full


# revision 15
# speedup vs baseline: 1.3141x; 1.3141x over previous
"""Cross-modal attention Trainium2 kernel (Bass/Tile), data-parallel over batch.

Per core (one batch element):
    q = img @ Wq.T + bq ; k = ts @ Wk.T + bk ; v = ts @ Wv.T + bv
    out = softmax(q @ k.T) @ v

Layout strategy: the host passes PRE-TRANSPOSED fp16 operands (imgT/tsT
[d, n], WqT/WkT/WvT [d, e]) so the contraction dim is already on SBUF
partitions when DMA'd in — no PE transposes, no staging casts, and half the
HBM traffic of fp32 (phase A is DMA-bound otherwise). fp16's 11-bit
mantissa keeps the unscaled scores (sigma~22) accurate enough; bf16 is not.
NOTE: never DMA with a float32r dtype — the DMA engages an fp32r rounding
mode whose per-engine state then corrupts unrelated fp16 XBAR-transpose
packets later in the kernel.
  - projections: fp16 matmuls, dc-outer accumulation (4 psum groups live)
    so the first matmuls start as soon as the first 128KB pieces land.
  - qT/kT [e%128, ec, n] written as fp32r by the ACT/DVE bias-add (legal
    fp32r producers); scores run on the PE in fp32r at bf16 rate.
  - v is produced naturally [j%128, jt, e] in fp16 for the PV matmul.
  - attention per q-tile: scores S[i=128, j=2048] as 2x [128,1024] PSUM
    chunk tiles; per-512 DVE reduce_max overlaps the score matmuls; exp on
    ACT (bias=-rowmax, accum_out=partial sums) written as fp16 probs; probs
    are DMA-XBAR-transposed into probsT [j, i] for the 16 fp16 PV matmuls;
    1/rowsum is applied to the PV result on ACT.
"""

import numpy as np

import concourse.bass as bass
import concourse.mybir as mybir
import concourse.tile as tile
from concourse import bacc
from concourse.bass_utils import run_bass_kernel_spmd

B, NQ, NK, D = 8, 2048, 2048, 512
P = 128
DC = D // P        # 4 contraction chunks
EC = D // P        # 4 output-dim chunks
TQ = NQ // P       # 16 query tiles
TK = NK // P       # 16 key tiles
JC = NK // 512     # 4 key chunks of 512 (scores free dim)
IC = NQ // 512     # 4 query chunks of 512 (projection free dim)

F32 = mybir.dt.float32
F32R = mybir.dt.float32r
F16 = mybir.dt.float16
AX = mybir.AxisListType.X
IDENT_FN = mybir.ActivationFunctionType.Identity
EXP_FN = mybir.ActivationFunctionType.Exp


def build():
    nc = bacc.Bacc(None, target_bir_lowering=False)

    imgT = nc.dram_tensor("imgT", [D, NQ], F16, kind="ExternalInput")
    tsT = nc.dram_tensor("tsT", [D, NK], F16, kind="ExternalInput")
    WqT = nc.dram_tensor("WqT", [D, D], F16, kind="ExternalInput")
    WkT = nc.dram_tensor("WkT", [D, D], F16, kind="ExternalInput")
    WvT = nc.dram_tensor("WvT", [D, D], F16, kind="ExternalInput")
    bq = nc.dram_tensor("bq", [D], F32, kind="ExternalInput")
    bk = nc.dram_tensor("bk", [D], F32, kind="ExternalInput")
    bv = nc.dram_tensor("bv", [D], F32, kind="ExternalInput")
    out = nc.dram_tensor("out", [NQ, D], F32, kind="ExternalOutput")

    with tile.TileContext(nc) as tc:
        with (
            tc.tile_pool(name="const", bufs=1) as const_pool,
            tc.tile_pool(name="big", bufs=1) as big,
        ):
            # biases: bq/bk as [P, EC] (e%128 on partitions), bv replicated
            # [P, D]; tiny descriptors go on the gpsimd SWDGE queue.
            bq_sb = const_pool.tile([P, EC], F32)
            nc.gpsimd.dma_start(bq_sb[:], bq.ap().rearrange("(c p) -> p c", p=P))
            bk_sb = const_pool.tile([P, EC], F32)
            nc.gpsimd.dma_start(bk_sb[:], bk.ap().rearrange("(c p) -> p c", p=P))
            bv_sb = const_pool.tile([P, D], F32)
            nc.gpsimd.dma_start(bv_sb[:], bv.ap().partition_broadcast(P))

            # persistent big operands
            qT = big.tile([P, EC, NQ], F32R)   # qT[p, ec, i] = q[i, ec*128+p]
            kT = big.tile([P, EC, NK], F32R)
            v_sb = big.tile([P, TK, D], F16)   # v_sb[p, jt, e] = v[jt*128+p, e]

            # ---- Phase A: projections (no transposes; dc-outer accumulate) ----
            with (
                tc.tile_pool(name="w", bufs=1) as wpool,
                tc.tile_pool(name="stage", bufs=4) as stage,
                tc.tile_pool(name="pps", bufs=8, space="PSUM") as pps,
            ):
                def emit_w(wdram, eng, idx):
                    # per-dc 128KB pieces so the first accumulation step can
                    # start before the whole matrix lands
                    w_sb = wpool.tile([P, DC, D], F16, name=f"w_{idx}")
                    for dc in range(DC):
                        eng.dma_start(w_sb[:, dc, :], wdram[dc * P : (dc + 1) * P, :])
                    return w_sb

                def emit_x(xdram, ic, eng, idx):
                    s = stage.tile([P, DC, 512], F16, tag="stage", name=f"s_{idx}")
                    for dc in range(DC):
                        eng.dma_start(
                            s[:, dc, :],
                            xdram[dc * P : (dc + 1) * P, ic * 512 : (ic + 1) * 512],
                        )
                    return s

                def emit_xproj(s, w_sb, dstT, bias_sb, ic):
                    """dstT[:, :, ic*512:+512] = W.T @ x chunk + bias, dc-outer."""
                    pqs = [
                        pps.tile([P, 512], F32, tag="pps", name=f"pq{ec}")
                        for ec in range(EC)
                    ]
                    for dc in range(DC):
                        for ec in range(EC):
                            nc.tensor.matmul(
                                pqs[ec][:],
                                w_sb[:, dc, ec * P : (ec + 1) * P],
                                s[:, dc, :],
                                start=(dc == 0),
                                stop=(dc == DC - 1),
                            )
                    for ec in range(EC):
                        if ec % 2 == 0:
                            nc.scalar.activation(
                                out=dstT[:, ec, ic * 512 : (ic + 1) * 512],
                                in_=pqs[ec][:],
                                func=IDENT_FN,
                                bias=bias_sb[:, ec : ec + 1],
                                scale=1.0,
                            )
                        else:
                            nc.vector.tensor_scalar_add(
                                dstT[:, ec, ic * 512 : (ic + 1) * 512],
                                pqs[ec][:],
                                bias_sb[:, ec : ec + 1],
                            )

                def emit_v(s, wv_sb, ic):
                    """v rows jt=4ic..4ic+3: v[j,e] = ts_chunk.T @ Wv.T + bv."""
                    for t in range(4):
                        jt = 4 * ic + t
                        pv = pps.tile([P, 512], F32, tag="pps")
                        for dc in range(DC):
                            nc.tensor.matmul(
                                pv[:],
                                s[:, dc, t * P : (t + 1) * P],
                                wv_sb[:, dc, :],
                                start=(dc == 0),
                                stop=(dc == DC - 1),
                            )
                        nc.vector.tensor_add(v_sb[:, jt, :], pv[:], bv_sb[:])

                # Two pure-DMA HWDGE queues; arrival order matches consumption
                # order so the PE never starves.
                wq_sb = emit_w(WqT, nc.sync, "q")
                s0 = emit_x(imgT, 0, nc.scalar, "s0")
                s1 = emit_x(imgT, 1, nc.sync, "s1")
                s2 = emit_x(imgT, 2, nc.scalar, "s2")
                emit_xproj(s0, wq_sb, qT, bq_sb, 0)
                s3 = emit_x(imgT, 3, nc.sync, "s3")
                wk_sb = emit_w(WkT, nc.scalar, "k")
                emit_xproj(s1, wq_sb, qT, bq_sb, 1)
                t0 = emit_x(tsT, 0, nc.sync, "t0")
                emit_xproj(s2, wq_sb, qT, bq_sb, 2)
                wv_sb = emit_w(WvT, nc.scalar, "v")
                emit_xproj(s3, wq_sb, qT, bq_sb, 3)

                t1 = emit_x(tsT, 1, nc.scalar, "t1")
                emit_xproj(t0, wk_sb, kT, bk_sb, 0)
                emit_v(t0, wv_sb, 0)
                t2 = emit_x(tsT, 2, nc.sync, "t2")
                emit_xproj(t1, wk_sb, kT, bk_sb, 1)
                emit_v(t1, wv_sb, 1)
                t3 = emit_x(tsT, 3, nc.scalar, "t3")
                emit_xproj(t2, wk_sb, kT, bk_sb, 2)
                emit_v(t2, wv_sb, 2)
                emit_xproj(t3, wk_sb, kT, bk_sb, 3)
                emit_v(t3, wv_sb, 3)

            # ---- Phase B: attention (software-pipelined by two q-tiles) ----
            with (
                tc.tile_pool(name="sps", bufs=3, space="PSUM") as sps,
                tc.tile_pool(name="ops", bufs=2, space="PSUM") as ops,
                tc.tile_pool(name="soft", bufs=3) as soft,
                tc.tile_pool(name="outp", bufs=2) as outp,
            ):
                stash = {}

                def emit_scores_softmax(qt):
                    pmax = soft.tile([P, JC], F32, tag="pmax")
                    chunks = []
                    for jc2 in range(2):
                        Sc = sps.tile([P, 1024], F32, tag="S")
                        chunks.append(Sc)
                        for h in range(2):
                            jc = 2 * jc2 + h
                            for ec in range(EC):
                                nc.tensor.matmul(
                                    Sc[:, h * 512 : (h + 1) * 512],
                                    qT[:, ec, qt * P : (qt + 1) * P],
                                    kT[:, ec, jc * 512 : (jc + 1) * 512],
                                    start=(ec == 0),
                                    stop=(ec == EC - 1),
                                )
                            # chunk max overlaps the next chunk's matmuls
                            nc.vector.reduce_max(
                                pmax[:, jc : jc + 1],
                                Sc[:, h * 512 : (h + 1) * 512],
                                axis=AX,
                            )
                    negmax = soft.tile([P, 1], F32, tag="negmax")
                    nc.vector.reduce_max(negmax[:], pmax[:], axis=AX, negate=True)
                    probs = soft.tile([P, NK], F16, tag="probs")
                    rowsum4 = soft.tile([P, 2], F32, tag="rowsum4")
                    for jc2 in range(2):
                        nc.scalar.activation(
                            out=probs[:, jc2 * 1024 : (jc2 + 1) * 1024],
                            in_=chunks[jc2][:],
                            func=EXP_FN,
                            bias=negmax[:],
                            scale=1.0,
                            accum_out=rowsum4[:, jc2 : jc2 + 1],
                        )
                    # transpose via the DMA XBAR (fp16): [i, j] -> [j%P, jt, i]
                    probsT = soft.tile([P, TK, P], F16, tag="probsT")
                    for jc2 in range(2):
                        nc.scalar.dma_start_transpose(
                            probsT[:, jc2 * 8 : (jc2 + 1) * 8, :],
                            probs[:, jc2 * 1024 : (jc2 + 1) * 1024],
                        )
                    stash[qt] = (probsT, rowsum4)

                def emit_pv(qt):
                    probsT, rowsum4 = stash.pop(qt)
                    rowsum = soft.tile([P, 1], F32, tag="rowsum")
                    nc.vector.reduce_sum(rowsum[:], rowsum4[:], axis=AX)
                    recip = soft.tile([P, 1], F32, tag="recip")
                    nc.vector.reciprocal(recip[:], rowsum[:])
                    po = ops.tile([P, D], F32, tag="po")
                    for jt in range(TK):
                        nc.tensor.matmul(
                            po[:],
                            probsT[:, jt, :],
                            v_sb[:, jt, :],
                            start=(jt == 0),
                            stop=(jt == TK - 1),
                        )
                    o_sb = outp.tile([P, D], F32, tag="o")
                    nc.scalar.mul(out=o_sb[:], in_=po[:], mul=recip[:])
                    nc.sync.dma_start(out[qt * P : (qt + 1) * P, :], o_sb[:])

                for qt in range(TQ):
                    emit_scores_softmax(qt)
                    if qt >= 2:
                        emit_pv(qt - 2)
                emit_pv(TQ - 2)
                emit_pv(TQ - 1)

    nc.compile()
    return nc


_NC_CACHE = None


def _get_nc():
    global _NC_CACHE
    if _NC_CACHE is None:
        _NC_CACHE = build()
    return _NC_CACHE


def run(inputs: dict, trace: bool = False):
    """Run on 8 cores, batch-parallel. Returns (out [B,NQ,D], BassKernelResults)."""
    nc = _get_nc()
    imgT = np.ascontiguousarray(
        np.transpose(np.asarray(inputs["img_feats"], np.float16), (0, 2, 1))
    )
    tsT = np.ascontiguousarray(
        np.transpose(np.asarray(inputs["ts_feats"], np.float16), (0, 2, 1))
    )
    WqT = np.ascontiguousarray(np.asarray(inputs["Wq"], np.float16).T)
    WkT = np.ascontiguousarray(np.asarray(inputs["Wk"], np.float16).T)
    WvT = np.ascontiguousarray(np.asarray(inputs["Wv"], np.float16).T)
    in_maps = []
    for b in range(B):
        in_maps.append(
            {
                "imgT": imgT[b],
                "tsT": tsT[b],
                "WqT": WqT,
                "WkT": WkT,
                "WvT": WvT,
                "bq": np.asarray(inputs["bq"], np.float32),
                "bk": np.asarray(inputs["bk"], np.float32),
                "bv": np.asarray(inputs["bv"], np.float32),
            }
        )
    res = run_bass_kernel_spmd(nc, in_maps, core_ids=list(range(B)), trace=trace)
    full = np.stack([res.results[b]["out"] for b in range(B)], axis=0)
    return full, res


def kernel(**inputs) -> np.ndarray:
    full, _ = run(inputs, trace=False)
    return full
